# revision 22
# baseline (speedup 1.0000x reference)
"""Trainium2 Bass kernel for a GPT-2 style transformer block
(S=3072, E=1024, 16 heads, MLP 4x), distributed over 8 NeuronCores.

v3 sharding (no AllGather):
  - LN1 runs sequence-parallel (each core normalizes+transposes its 384-row
    chunk into [E, CH] bf16).
  - Each core computes q/k/v for ALL heads on its own chunk from the full
    qkv weights, then three pipelined AllToAlls (k, q, v) reshard to
    tensor-parallel-by-head: core h ends with q/k [128 dims of its 2 heads,
    full S] and v [full S, 128 dims].  This replaces the 72us AllGather of
    normalized activations with ~3 x ~12us A2As that overlap qkv compute.
  - v is computed directly in ROW layout ([keys, dims]) so no PE transposes
    are needed to build the AV stationary operand.
  - Attention is tensor-parallel over heads (2 heads/core), h-outer so the
    head-0 output AllToAll overlaps head-1 compute.
  - Scores matmuls (K=64) are packed two-at-a-time into PE quadrants via
    tile_position (keys split 64/64), doubling score throughput.
  - Softmax exp runs on 3-key-block groups ([128, 3*384] via 3D AP over a
    3-bank PSUM tile) to amortize the ~250ns ScalarE per-instruction bubble.
  - The external mask is applied multiplicatively to v_aug (masked keys zero
    both numerator and denominator contributions); exp needs no bias.
  - Softmax denominator comes free from 64 ones-rows in v_aug; normalization
    uses reciprocal_approx_fast (the exact RECIPROCAL was 2.5us/instr).
  - proj + residual + LN2 + full MLP run per-chunk as before.
  - A tiny warmup AllToAll fires at t=0 so CC rendezvous overlaps LN1.
"""

import numpy as np

E, H, I = 1024, 16, 4096
W = 8

PACK_SCORES = True   # tile_position quadrant packing for K=64 score matmuls
EXP_GROUP = True     # single 3D-AP exp per 3-key-block group

_CACHE = {}


def _build(SS: int, dt_name: str = "bfloat16", mock_cc: bool = False,
           debug: bool = False):
    """Build the SPMD Bass program for sequence length SS.
    dt_name: 'bfloat16' (fast) or 'float32' (exact, sim-only)."""
    import concourse.mybir as mybir
    import concourse.tile as tile
    from concourse import bacc
    from concourse.masks import make_identity

    f32 = mybir.dt.float32
    dt = getattr(mybir.dt, dt_name)
    AF = mybir.ActivationFunctionType
    ALU = mybir.AluOpType
    X = mybir.AxisListType.X

    CH = SS // W          # seq rows per core
    NB = SS // 128        # 128-blocks along full sequence
    B = CH // 128         # 128-blocks per chunk

    # f32 (sim) tiles are 2x the size; shrink buffering to fit SBUF there.
    # debug builds carry a 10KB dbg tile, so shrink MLP streaming bufs there.
    sim = dt_name == "float32"
    QKW_BUFS = 1 if sim else 2
    W1M_BUFS = 2 if (sim or debug) else 4
    W2M_BUFS = 1 if (sim or debug) else 2
    WPM_BUFS = 2 if sim else 3
    LN_BUFS = 1 if sim else 2
    VR_BUFS = 1 if sim else 2
    OROW_BUFS = 1 if sim else 2

    nc = bacc.Bacc(None)

    hid = nc.dram_tensor("hidden", [CH, E], f32, kind="ExternalInput")
    # q,k stationary weight blocks, m-major: m 0..7 = q out-blocks (prescaled
    # 1/8), m 8..15 = k out-blocks; within m: 8 contraction blocks of [128,128]
    qkw = nc.dram_tensor("qkw", [128, 24 * 8 * 128], dt, kind="ExternalInput")
    qkb = nc.dram_tensor("qkb", [128, 24], f32, kind="ExternalInput")
    mask01 = nc.dram_tensor("mask01", [128, NB], f32, kind="ExternalInput")
    proj_w = nc.dram_tensor("proj_w", [8 * 128, 8 * 128], dt,
                            kind="ExternalInput")
    proj_b = nc.dram_tensor("proj_b", [128, 8], f32, kind="ExternalInput")
    ln1_w = nc.dram_tensor("ln1_w", [128, 8], f32, kind="ExternalInput")
    ln1_b = nc.dram_tensor("ln1_b", [128, 8], f32, kind="ExternalInput")
    ln2_w = nc.dram_tensor("ln2_w", [128, 8], f32, kind="ExternalInput")
    ln2_b = nc.dram_tensor("ln2_b", [128, 8], f32, kind="ExternalInput")
    w1 = nc.dram_tensor("w1", [32 * 128, 8 * 128], dt, kind="ExternalInput")
    b1 = nc.dram_tensor("b1", [128, 32], f32, kind="ExternalInput")
    w2 = nc.dram_tensor("w2", [8 * 128, 32 * 128], dt, kind="ExternalInput")
    b2 = nc.dram_tensor("b2", [128, 8], f32, kind="ExternalInput")
    cbm_in = nc.dram_tensor("cbm", [128, B * CH], dt, kind="ExternalInput")
    out = nc.dram_tensor("out", [CH, E], f32, kind="ExternalOutput")
    dbg_t = (nc.dram_tensor("dbg", [128, 10 * 256], f32, kind="ExternalOutput")
             if debug else None)

    rg = [list(range(W))]

    with tile.TileContext(nc) as tc:
        with (
            tc.tile_pool(name="dram", bufs=1, space="DRAM") as dram,
            tc.tile_pool(name="const", bufs=1) as const,
            tc.tile_pool(name="persist", bufs=1) as persist,
            tc.tile_pool(name="work", bufs=2) as work,
            tc.tile_pool(name="exp", bufs=3) as exp_pool,
            # PSUM: tag "sc" = 3 banks x2 bufs, tag "av" = 1 bank x2 bufs
            tc.tile_pool(name="psum", bufs=2, space="PSUM") as psum,
        ):
            def dma(out_, in_):
                return nc.sync.dma_start(out=out_, in_=in_)

            dbg_sb = (persist.tile([128, 10 * 256], f32, tag="dbg",
                                   name="dbg") if debug else None)

            def dbg(slot, src_fn, rows=128):
                if debug:
                    nc.vector.tensor_copy(
                        dbg_sb[0:rows, slot * 256:(slot + 1) * 256], src_fn())

            # ----- warmup collective: CC rendezvous overlaps LN1 -----
            wu_in = dram.tile([W, 16], f32, name="wu_in")
            wu_out = dram.tile([W, 16], f32, name="wu_out")
            if not mock_cc:
                if sim:
                    nc.sync.dma_start(out=wu_in[:, :], in_=qkb[0:W, 0:16])
                nc.gpsimd.collective_compute(
                    "AllToAll", ALU.bypass, replica_groups=rg,
                    ins=[wu_in.opt()], outs=[wu_out.opt()])

            # ----- constants -----
            ident = const.tile([128, 128], f32, tag="ident", name="ident")
            make_identity(nc, ident[:])
            ident_h = const.tile([128, 128], dt, tag="identh", name="identh")
            nc.vector.tensor_copy(ident_h[:], ident[:])
            eps_sb = const.tile([128, 1], f32, tag="eps", name="eps")
            nc.vector.memset(eps_sb[:], 1e-5)

            cbm_all = const.tile([128, B * CH], dt, tag="cbm", name="cbm")
            dma(cbm_all[:], cbm_in[:, :])

            def load2d(dram_t, shape, name):
                t = const.tile(shape, f32, tag=name, name=name)
                dma(t[:], dram_t[:, :])
                return t

            ln1_w_sb = load2d(ln1_w, [128, 8], "ln1w")
            ln1_b_sb = load2d(ln1_b, [128, 8], "ln1b")
            ln2_w_sb = load2d(ln2_w, [128, 8], "ln2w")
            ln2_b_sb = load2d(ln2_b, [128, 8], "ln2b")
            qkb_sb = load2d(qkb, [128, 24], "qkb")
            proj_b_sb = load2d(proj_b, [128, 8], "projb")
            b1_sb = load2d(b1, [128, 32], "b1")
            b2_sb = load2d(b2, [128, 8], "b2")
            m01_sb = load2d(mask01, [128, NB], "m01")

            # ----- weights: k-half of qkw first (needed first), q-half after
            # (reuses the same slot via tag); wv for the v rows -----
            kw_sb = persist.tile([128, 8 * 8 * 128], dt, tag="qkw", bufs=QKW_BUFS,
                                 name="kw")
            dma(kw_sb[:], qkw[:, 8 * 8 * 128:16 * 8 * 128])  # k blocks

            # ----- LN (row layout) + transpose into [128, 8*CH] bf16 tile -----
            def layer_norm_T(x_tiles, w_sb, b_sb, out_all):
                for t in range(len(x_tiles)):
                    x = x_tiles[t]
                    stat = work.tile([128, 8], f32, tag="lnstat", name="lnstat")
                    scr = work.tile([128, E], dt, tag="lnscr", bufs=LN_BUFS, name="lnscr")
                    nc.vector.reduce_sum(out=stat[:, 0:1], in_=x[:], axis=X)
                    nc.scalar.activation(scr[:], x[:], AF.Square,
                                         accum_out=stat[:, 1:2])
                    nc.vector.tensor_tensor(out=stat[:, 4:5], in0=stat[:, 0:1],
                                            in1=stat[:, 0:1], op=ALU.mult)
                    nc.vector.tensor_scalar(out=stat[:, 4:5], in0=stat[:, 4:5],
                                            scalar1=-1.0 / E,
                                            scalar2=stat[:, 1:2],
                                            op0=ALU.mult, op1=ALU.add)
                    nc.scalar.activation(stat[:, 5:6], stat[:, 4:5], AF.Sqrt,
                                         bias=eps_sb[:], scale=1.0 / E)
                    nc.vector.reciprocal_approx_fast(out=stat[:, 3:4],
                                                     in_=stat[:, 5:6])
                    nc.vector.tensor_scalar(out=stat[:, 6:7], in0=stat[:, 0:1],
                                            scalar1=stat[:, 3:4],
                                            scalar2=-1.0 / E,
                                            op0=ALU.mult, op1=ALU.mult)
                    xn = work.tile([128, E], dt, tag="lnxn", bufs=LN_BUFS, name="lnxn")
                    nc.vector.tensor_scalar(out=xn[:], in0=x[:],
                                            scalar1=stat[:, 3:4],
                                            scalar2=stat[:, 6:7],
                                            op0=ALU.mult, op1=ALU.add)
                    for m in range(8):
                        tp = psum.tile([128, 128], dt, tag="sc", name="tp")
                        nc.tensor.transpose(tp[:], xn[:, m * 128:(m + 1) * 128],
                                            ident_h[:])
                        dst = out_all[:, m * CH + t * 128: m * CH + (t + 1) * 128]
                        if m % 2 == 0:
                            nc.scalar.activation(dst, tp[:], AF.Identity,
                                                 bias=b_sb[:, m:m + 1],
                                                 scale=w_sb[:, m:m + 1])
                        else:
                            nc.vector.tensor_scalar(
                                out=dst, in0=tp[:],
                                scalar1=w_sb[:, m:m + 1],
                                scalar2=b_sb[:, m:m + 1],
                                op0=ALU.mult, op1=ALU.add)

            # ----- stage 1: LN1 on own chunk -----
            x_rows = []
            for t in range(B):
                xt = work.tile([128, E], f32, tag="xrow", bufs=3,
                               name=f"xrow{t}")
                dma(xt[:], hid[t * 128:(t + 1) * 128, :])
                x_rows.append(xt)
            xnT = persist.tile([128, 8 * CH], dt, tag="xnT", name="xnT")
            layer_norm_T(x_rows, ln1_w_sb, ln1_b_sb, xnT)
            dbg(0, lambda: xnT[:, 0:256])

            # ----- stage 2: local qkv (all heads) + A2As -----
            a2a_k_in = dram.tile([W * 128, CH], dt, name="a2a_k_in")
            a2a_k_out = dram.tile([W * 128, CH], dt, name="a2a_k_out")
            a2a_q_in = dram.tile([W * 128, CH], dt, name="a2a_q_in")
            a2a_q_out = dram.tile([W * 128, CH], dt, name="a2a_q_out")
            a2a_v_in = dram.tile([W * 128, CH], dt, name="a2a_v_in")
            a2a_v_out = dram.tile([W * 128, CH], dt, name="a2a_v_out")

            def qk_part(w_sb, boff, a2a_in):
                for m in range(8):
                    ps = psum.tile([128, CH], f32, tag="av", name="qkps")
                    for kb in range(8):
                        nc.tensor.matmul(
                            ps[:],
                            lhsT=w_sb[:, (m * 8 + kb) * 128:(m * 8 + kb + 1) * 128],
                            rhs=xnT[:, kb * CH:(kb + 1) * CH],
                            start=(kb == 0), stop=(kb == 7))
                    sb = work.tile([128, CH], dt, tag="qkloc", bufs=3,
                                   name="qkloc")
                    nc.vector.tensor_scalar(
                        out=sb[:], in0=ps[:],
                        scalar1=qkb_sb[:, boff + m:boff + m + 1],
                        scalar2=None, op0=ALU.add)
                    dma(a2a_in[m * 128:(m + 1) * 128, :], sb[:])

            qk_part(kw_sb, 8, a2a_k_in)
            if mock_cc:
                dma(a2a_k_out[:, :], a2a_k_in[:, :])
            else:
                nc.gpsimd.collective_compute(
                    "AllToAll", ALU.bypass, replica_groups=rg,
                    ins=[a2a_k_in.opt()], outs=[a2a_k_out.opt()])

            qw_sb = persist.tile([128, 8 * 8 * 128], dt, tag="qkw", bufs=QKW_BUFS,
                                 name="qw")
            dma(qw_sb[:], qkw[:, 0:8 * 8 * 128])
            qk_part(qw_sb, 0, a2a_q_in)
            if mock_cc:
                dma(a2a_q_out[:, :], a2a_q_in[:, :])
            else:
                nc.gpsimd.collective_compute(
                    "AllToAll", ALU.bypass, replica_groups=rg,
                    ins=[a2a_q_in.opt()], outs=[a2a_q_out.opt()])

            # v part: same T-layout as k/q (v weight blocks 16..23 reuse slot)
            vw_sb = persist.tile([128, 8 * 8 * 128], dt, tag="qkw",
                                 bufs=QKW_BUFS, name="vw")
            dma(vw_sb[:], qkw[:, 16 * 8 * 128:24 * 8 * 128])
            qk_part(vw_sb, 16, a2a_v_in)
            if mock_cc:
                dma(a2a_v_out[:, :], a2a_v_in[:, :])
            else:
                nc.gpsimd.collective_compute(
                    "AllToAll", ALU.bypass, replica_groups=rg,
                    ins=[a2a_v_in.opt()], outs=[a2a_v_out.opt()])

            # ----- stage 3: assemble kT/qT [128, SS] and v_aug -----
            kT = persist.tile([128, SS], dt, tag="kT", name="kT")
            qT = persist.tile([128, SS], dt, tag="qT", name="qT")
            for s in range(W):
                nc.scalar.dma_start(out=kT[:, s * CH:(s + 1) * CH],
                                    in_=a2a_k_out[s * 128:(s + 1) * 128, :])
                nc.scalar.dma_start(out=qT[:, s * CH:(s + 1) * CH],
                                    in_=a2a_q_out[s * 128:(s + 1) * 128, :])

            # v_aug: [key 128, block*128] rows; cols 0:64 = v dims, 64:128 ones
            # then whole block scaled by external mask (masked keys zero both
            # numerator and denominator => correct softmax masking)
            v_aug = [persist.tile([128, NB * 128], dt, tag=f"vaug{h}",
                                  name=f"vaug{h}") for h in range(2)]
            for h in range(2):
                nc.vector.memset(
                    v_aug[h][:].rearrange("p (b c) -> p b c", c=128)[:, :, 64:128],
                    1.0)
            for tb in range(NB):
                s, r = tb // B, tb % B
                for h in range(2):
                    vsrc = a2a_v_out[s * 128 + 64 * h:s * 128 + 64 * h + 64,
                                     r * 128:(r + 1) * 128]
                    vdst = v_aug[h][:, tb * 128:tb * 128 + 64]
                    if sim:
                        nc.sync.dma_start(out=vdst,
                                          in_=vsrc.rearrange("a b -> b a"))
                    else:
                        nc.sync.dma_start_transpose(vdst, vsrc)
                    nc.vector.tensor_scalar(
                        out=v_aug[h][:, tb * 128:(tb + 1) * 128],
                        in0=v_aug[h][:, tb * 128:(tb + 1) * 128],
                        scalar1=m01_sb[:, tb:tb + 1],
                        scalar2=None, op0=ALU.mult)

            dbg(1, lambda: kT[:, 0:256])
            dbg(2, lambda: qT[:, 0:256])
            dbg(3, lambda: v_aug[0][:, 0:256])

            # ----- stage 4/5: attention, heads interleaved per group so the
            # PE always has a backlog (other head's scores+AV) while ScalarE
            # runs exp — keeps HAM at full clock.  Single combined out A2A.
            a2a_o_in = dram.tile([W * 128, CH], dt, name="a2a_o_in")
            a2a_o_out = dram.tile([W * 128, CH], dt, name="a2a_o_out")
            for j in range(W):
                n_t = B * (j + 1)
                avs = [psum.tile([128, CH], f32, tag="av", name=f"avacc{h}")
                       for h in range(2)]
                for g0 in range(0, n_t, 3):
                    g = min(3, n_t - g0)
                    for h in range(2):
                        e0 = 64 * h
                        av = avs[h]
                        sc = psum.tile([128, 3 * 512], f32, tag="sc",
                                       name="sc")
                        for s in range(g):
                            tb = g0 + s
                            if PACK_SCORES:
                                for hf in range(2):
                                    nc.tensor.matmul(
                                        sc[64 * hf:64 * hf + 64,
                                           s * 512:s * 512 + CH],
                                        lhsT=kT[e0:e0 + 64,
                                                tb * 128 + 64 * hf:tb * 128 + 64 * hf + 64],
                                        rhs=qT[e0:e0 + 64, j * CH:(j + 1) * CH],
                                        start=True, stop=True,
                                        tile_position=(e0, 64 * hf))
                            else:
                                nc.tensor.matmul(
                                    sc[:, s * 512:s * 512 + CH],
                                    lhsT=kT[e0:e0 + 64,
                                            tb * 128:(tb + 1) * 128],
                                    rhs=qT[e0:e0 + 64, j * CH:(j + 1) * CH],
                                    start=True, stop=True)
                        ex = exp_pool.tile([128, 3 * CH], dt, tag="ex",
                                           name="ex")
                        if EXP_GROUP:
                            sc3 = sc[:].rearrange("p (s c) -> p s c", c=512)
                            ex3 = ex[:].rearrange("p (s c) -> p s c", c=CH)
                            nc.scalar.activation(ex3[:, 0:g, 0:CH],
                                                 sc3[:, 0:g, 0:CH], AF.Exp)
                        else:
                            for s in range(g):
                                nc.scalar.activation(
                                    ex[:, s * CH:(s + 1) * CH],
                                    sc[:, s * 512:s * 512 + CH], AF.Exp)
                        if g0 + g == n_t:
                            # diagonal band: zero future lanes (last B blocks)
                            nc.vector.tensor_tensor(
                                out=ex[:, (g - B) * CH:g * CH],
                                in0=ex[:, (g - B) * CH:g * CH],
                                in1=cbm_all[:], op=ALU.mult)
                        if h == 0 and j == 0 and g0 == 0:
                            dbg(4, lambda ex=ex: ex[:, 0:256])
                        for s in range(g):
                            tb = g0 + s
                            nc.tensor.matmul(
                                av[:],
                                lhsT=v_aug[h][:, tb * 128:(tb + 1) * 128],
                                rhs=ex[:, s * CH:(s + 1) * CH],
                                start=(tb == 0), stop=(tb == n_t - 1))
                for h in range(2):
                    av = avs[h]
                    den = work.tile([64, CH], f32, tag="den", name="den")
                    nc.vector.tensor_copy(den[:], av[64:128, :])
                    rec = work.tile([64, CH], f32, tag="rec", name="rec")
                    nc.vector.reciprocal_approx_fast(out=rec[:], in_=den[:])
                    avn = work.tile([64, CH], dt, tag="avn", name="avn")
                    nc.vector.tensor_tensor(out=avn[:], in0=av[0:64, :],
                                            in1=rec[:], op=ALU.mult)
                    if h == 0 and j == 0:
                        dbg(5, lambda avn=avn: avn[:, 0:256], rows=64)
                    dma(a2a_o_in[j * 128 + 64 * h:j * 128 + 64 * h + 64, :],
                        avn[:])
            if mock_cc:
                dma(a2a_o_out[:, :], a2a_o_in[:, :])
            else:
                nc.gpsimd.collective_compute(
                    "AllToAll", ALU.bypass, replica_groups=rg,
                    ins=[a2a_o_in.opt()], outs=[a2a_o_out.opt()])

            # ----- stage 7: proj + residual -----
            aT = persist.tile([128, SS], dt, tag="aT", name="aT")
            for k in range(W):
                nc.scalar.dma_start(
                    out=aT[:, k * CH:(k + 1) * CH],
                    in_=a2a_o_out[k * 128:(k + 1) * 128, :])
            dbg(6, lambda: aT[:, 0:256])
            res1 = [persist.tile([128, E], f32, tag=f"res1_{t}",
                                 name=f"res1_{t}") for t in range(B)]
            for m in range(8):
                wpm = work.tile([128, 8 * 128], dt, tag="wpm", name="wpm",
                                bufs=WPM_BUFS)
                dma(wpm[:], proj_w[m * 128:(m + 1) * 128, :])
                ps = psum.tile([128, CH], f32, tag="av", name="mmacc")
                for k in range(8):
                    nc.tensor.matmul(
                        ps[:], lhsT=wpm[:, k * 128:(k + 1) * 128],
                        rhs=aT[:, k * CH:(k + 1) * CH],
                        start=(k == 0), stop=(k == 7))
                pTm = work.tile([128, CH], f32, tag="pTm", name="pTm")
                nc.scalar.activation(pTm[:], ps[:], AF.Identity,
                                     bias=proj_b_sb[:, m:m + 1], scale=1.0)
                for t in range(B):
                    xr = work.tile([128, 128], f32, tag="xres", bufs=4,
                                   name="xres")
                    dma(xr[:], hid[t * 128:(t + 1) * 128,
                                   m * 128:(m + 1) * 128])
                    tp = psum.tile([128, 128], f32, tag="sc", name="tp")
                    nc.tensor.transpose(tp[:], pTm[:, t * 128:(t + 1) * 128],
                                        ident[:])
                    nc.vector.tensor_tensor(
                        out=res1[t][:, m * 128:(m + 1) * 128],
                        in0=tp[:], in1=xr[:], op=ALU.add)

            # ----- stage 8: LN2 -----
            dbg(7, lambda: res1[0][:, 0:256])
            l2T = persist.tile([128, 8 * CH], dt, tag="l2T", name="l2T")
            layer_norm_T(res1, ln2_w_sb, ln2_b_sb, l2T)
            dbg(8, lambda: l2T[:, 0:256])

            # ----- stage 9: MLP -----
            scratch = persist.tile([128, 40 * CH], dt, tag="scratch",
                                   name="scratch")
            h1T = [scratch[:, m * CH:(m + 1) * CH] for m in range(32)]
            for m in range(32):
                w1m = work.tile([128, E], dt, tag="w1m", name="w1m", bufs=W1M_BUFS)
                dma(w1m[:], w1[m * 128:(m + 1) * 128, :])
                ps = psum.tile([128, CH], f32, tag="av", name="mmacc")
                for k in range(8):
                    nc.tensor.matmul(
                        ps[:], lhsT=w1m[:, k * 128:(k + 1) * 128],
                        rhs=l2T[:, k * CH:(k + 1) * CH],
                        start=(k == 0), stop=(k == 7))
                nc.scalar.activation(h1T[m], ps[:], AF.Relu,
                                     bias=b1_sb[:, m:m + 1], scale=1.0)

            oT = [scratch[:, (32 + m) * CH:(33 + m) * CH] for m in range(8)]
            for m in range(8):
                ps = psum.tile([128, CH], f32, tag="av", name="mmacc")
                for half in range(2):
                    w2m = work.tile([128, 16 * 128], dt, tag="w2m", name="w2m",
                                    bufs=W2M_BUFS)
                    dma(w2m[:], w2[m * 128:(m + 1) * 128,
                                   half * 16 * 128:(half + 1) * 16 * 128])
                    for k in range(16):
                        nc.tensor.matmul(
                            ps[:], lhsT=w2m[:, k * 128:(k + 1) * 128],
                            rhs=h1T[half * 16 + k],
                            start=(half == 0 and k == 0),
                            stop=(half == 1 and k == 15))
                nc.scalar.activation(oT[m], ps[:], AF.Identity,
                                     bias=b2_sb[:, m:m + 1], scale=1.0)

            # ----- stage 10: transpose back + final residual + out -----
            for t in range(B):
                orow = work.tile([128, E], f32, tag="orow", bufs=OROW_BUFS, name="orow")
                for m in range(8):
                    tp = psum.tile([128, 128], dt, tag="sc", name="tpo")
                    nc.tensor.transpose(tp[:], oT[m][:, t * 128:(t + 1) * 128],
                                        ident_h[:])
                    nc.vector.tensor_tensor(
                        out=orow[:, m * 128:(m + 1) * 128],
                        in0=tp[:], in1=res1[t][:, m * 128:(m + 1) * 128],
                        op=ALU.add)
                dma(out[t * 128:(t + 1) * 128, :], orow[:])
            if debug:
                dbg(9, lambda: oT[0][:, 0:256])
                dma(dbg_t[:, :], dbg_sb[:])

    return nc


def _cbm(CH, wdt):
    Bv = CH // 128
    t = np.arange(128)[:, None]
    s = np.arange(CH)[None, :]
    cb = np.zeros((128, Bv * CH), np.float32)
    for p in range(Bv):
        cb[:, p * CH:(p + 1) * CH] = (s - t - 128 * p >= 0)
    return np.ascontiguousarray(cb.astype(wdt))


def _prepare_in_maps(inputs, SS: int, dt_name: str = "bfloat16"):
    """Host-side prep: slice hidden per core; pre-tile weight matrices so
    every device DMA is contiguous; cast mm weights to dt; prescale q 1/8."""
    import ml_dtypes

    wdt = ml_dtypes.bfloat16 if dt_name == "bfloat16" else np.float32
    CH = SS // W
    NB = SS // 128
    hid = np.ascontiguousarray(
        np.asarray(inputs["hidden_states"], np.float32)[0, :SS])
    attn_w = np.asarray(inputs["attn_w"], np.float32).copy()
    attn_b = np.asarray(inputs["attn_b"], np.float32).copy()
    attn_w[:, :E] *= 0.125
    attn_b[:E] *= 0.125
    mask = np.asarray(inputs["mask"])[0, 0, 0, :SS]
    mask01 = mask.astype(np.float32)

    def vec2d(v, n):
        return np.ascontiguousarray(
            np.asarray(v, np.float32)[:n].reshape(n // 128, 128).T)

    proj_w = np.asarray(inputs["proj_w"], np.float32)
    w1 = np.asarray(inputs["mlp_w1"], np.float32)
    w2 = np.asarray(inputs["mlp_w2"], np.float32)

    # X[k*128+p, m*128+f] -> [(m p), (k f)]
    def tile_mk(x, km, mm_):
        return np.ascontiguousarray(
            x.reshape(km, 128, mm_, 128).transpose(2, 1, 0, 3)
            .reshape(mm_ * 128, km * 128))

    # q,k,v stationary blocks: qkw[:, (m*8+kb)*128 : +128] =
    #   attn_w[kb*128:(kb+1)*128, c0 + m'*128 : +128]
    # m 0..7 = q (prescaled 1/8), 8..15 = k, 16..23 = v
    qkw = np.empty((128, 24, 8, 128), np.float32)
    qkb = np.empty((128, 24), np.float32)
    for m in range(24):
        c0 = (m // 8) * E
        mm = m % 8
        for kb in range(8):
            qkw[:, m, kb, :] = attn_w[kb * 128:(kb + 1) * 128,
                                      c0 + mm * 128:c0 + (mm + 1) * 128]
        qkb[:, m] = attn_b[c0 + mm * 128:c0 + (mm + 1) * 128]

    common = {
        "qkw": np.ascontiguousarray(qkw.reshape(128, -1)).astype(wdt),
        "qkb": np.ascontiguousarray(qkb),
        "mask01": np.ascontiguousarray(mask01.reshape(NB, 128).T),
        "proj_w": tile_mk(proj_w, 8, 8).astype(wdt),
        "proj_b": vec2d(inputs["proj_b"], E),
        "ln1_w": vec2d(inputs["ln1_w"], E),
        "ln1_b": vec2d(inputs["ln1_b"], E),
        "ln2_w": vec2d(inputs["ln2_w"], E),
        "ln2_b": vec2d(inputs["ln2_b"], E),
        "w1": tile_mk(w1, 8, 32).astype(wdt),
        "b1": vec2d(inputs["mlp_b1"], I),
        "w2": tile_mk(w2, 32, 8).astype(wdt),
        "b2": vec2d(inputs["mlp_b2"], E),
        "cbm": _cbm(CH, wdt),
    }
    in_maps = []
    for i in range(W):
        in_maps.append({
            "hidden": np.ascontiguousarray(hid[i * CH:(i + 1) * CH]),
            **common,
        })
    return in_maps


def _run(inputs, SS, dt_name="bfloat16", **kw):
    from concourse.bass_utils import run_bass_kernel_spmd

    key = (SS, dt_name)
    if key not in _CACHE:
        nc = _build(SS, dt_name)
        nc.finalize()
        _CACHE[key] = nc
    nc = _CACHE[key]
    in_maps = _prepare_in_maps(inputs, SS, dt_name)
    res = run_bass_kernel_spmd(nc, in_maps, core_ids=list(range(W)), **kw)
    full = np.concatenate([r["out"] for r in res.results], axis=0)
    return full[None].astype(np.float32), res


def kernel(**inputs) -> np.ndarray:
    out, _ = _run(inputs, 3072, "bfloat16")
    return out


# revision 24
# speedup vs baseline: 1.0958x; 1.0958x over previous
"""Trainium2 Bass kernel for a GPT-2 style transformer block
(S=3072, E=1024, 16 heads, MLP 4x), distributed over 8 NeuronCores.

v3 sharding (no AllGather):
  - LN1 runs sequence-parallel (each core normalizes+transposes its 384-row
    chunk into [E, CH] bf16).
  - Each core computes q/k/v for ALL heads on its own chunk from the full
    qkv weights, then three pipelined AllToAlls (k, q, v) reshard to
    tensor-parallel-by-head: core h ends with q/k [128 dims of its 2 heads,
    full S] and v [full S, 128 dims].  This replaces the 72us AllGather of
    normalized activations with ~3 x ~12us A2As that overlap qkv compute.
  - v is computed directly in ROW layout ([keys, dims]) so no PE transposes
    are needed to build the AV stationary operand.
  - Attention is tensor-parallel over heads (2 heads/core), h-outer so the
    head-0 output AllToAll overlaps head-1 compute.
  - Scores matmuls (K=64) are packed two-at-a-time into PE quadrants via
    tile_position (keys split 64/64), doubling score throughput.
  - Softmax exp runs on 3-key-block groups ([128, 3*384] via 3D AP over a
    3-bank PSUM tile) to amortize the ~250ns ScalarE per-instruction bubble.
  - The external mask is applied multiplicatively to v_aug (masked keys zero
    both numerator and denominator contributions); exp needs no bias.
  - Softmax denominator comes free from 64 ones-rows in v_aug; normalization
    uses reciprocal_approx_fast (the exact RECIPROCAL was 2.5us/instr).
  - proj + residual + LN2 + full MLP run per-chunk as before.
  - A tiny warmup AllToAll fires at t=0 so CC rendezvous overlaps LN1.
"""

import numpy as np

E, H, I = 1024, 16, 4096
W = 8

PACK_SCORES = True   # tile_position quadrant packing for K=64 score matmuls
EXP_GROUP = True     # single 3D-AP exp per 3-key-block group

_CACHE = {}


def _build(SS: int, dt_name: str = "bfloat16", mock_cc: bool = False,
           debug: bool = False):
    """Build the SPMD Bass program for sequence length SS.
    dt_name: 'bfloat16' (fast) or 'float32' (exact, sim-only)."""
    import concourse.mybir as mybir
    import concourse.tile as tile
    from concourse import bacc
    from concourse.masks import make_identity

    f32 = mybir.dt.float32
    dt = getattr(mybir.dt, dt_name)
    AF = mybir.ActivationFunctionType
    ALU = mybir.AluOpType
    X = mybir.AxisListType.X

    CH = SS // W          # seq rows per core
    NB = SS // 128        # 128-blocks along full sequence
    B = CH // 128         # 128-blocks per chunk

    # f32 (sim) tiles are 2x the size; shrink buffering to fit SBUF there.
    # debug builds carry a 10KB dbg tile, so shrink MLP streaming bufs there.
    sim = dt_name == "float32"
    QKW_BUFS = 1 if sim else 2
    W1M_BUFS = 2 if (sim or debug) else 4
    W2M_BUFS = 1 if (sim or debug) else 2
    WPM_BUFS = 2 if sim else 3
    LN_BUFS = 1 if sim else 2
    VR_BUFS = 1 if sim else 2
    OROW_BUFS = 1 if sim else 2

    nc = bacc.Bacc(None)

    hid = nc.dram_tensor("hidden", [CH, E], f32, kind="ExternalInput")
    # q,k stationary weight blocks, m-major: m 0..7 = q out-blocks (prescaled
    # 1/8), m 8..15 = k out-blocks; within m: 8 contraction blocks of [128,128]
    qkw = nc.dram_tensor("qkw", [128, 24 * 8 * 128], dt, kind="ExternalInput")
    qkb = nc.dram_tensor("qkb", [128, 24], f32, kind="ExternalInput")
    mask01 = nc.dram_tensor("mask01", [128, NB], f32, kind="ExternalInput")
    proj_w = nc.dram_tensor("proj_w", [8 * 128, 8 * 128], dt,
                            kind="ExternalInput")
    proj_b = nc.dram_tensor("proj_b", [128, 8], f32, kind="ExternalInput")
    ln1_w = nc.dram_tensor("ln1_w", [128, 8], f32, kind="ExternalInput")
    ln1_b = nc.dram_tensor("ln1_b", [128, 8], f32, kind="ExternalInput")
    ln2_w = nc.dram_tensor("ln2_w", [128, 8], f32, kind="ExternalInput")
    ln2_b = nc.dram_tensor("ln2_b", [128, 8], f32, kind="ExternalInput")
    w1 = nc.dram_tensor("w1", [32 * 128, 8 * 128], dt, kind="ExternalInput")
    b1 = nc.dram_tensor("b1", [128, 32], f32, kind="ExternalInput")
    w2 = nc.dram_tensor("w2", [8 * 128, 32 * 128], dt, kind="ExternalInput")
    b2 = nc.dram_tensor("b2", [128, 8], f32, kind="ExternalInput")
    cbm_in = nc.dram_tensor("cbm", [128, B * CH], dt, kind="ExternalInput")
    out = nc.dram_tensor("out", [CH, E], f32, kind="ExternalOutput")
    dbg_t = (nc.dram_tensor("dbg", [128, 10 * 256], f32, kind="ExternalOutput")
             if debug else None)

    rg = [list(range(W))]

    with tile.TileContext(nc) as tc:
        with (
            tc.tile_pool(name="dram", bufs=1, space="DRAM") as dram,
            tc.tile_pool(name="const", bufs=1) as const,
            tc.tile_pool(name="persist", bufs=1) as persist,
            tc.tile_pool(name="work", bufs=2) as work,
            tc.tile_pool(name="exp", bufs=3) as exp_pool,
            # PSUM: tag "sc" = 3 banks x2 bufs, tag "av" = 1 bank x2 bufs
            tc.tile_pool(name="psum", bufs=2, space="PSUM") as psum,
        ):
            def dma(out_, in_):
                return nc.sync.dma_start(out=out_, in_=in_)

            dbg_sb = (persist.tile([128, 10 * 256], f32, tag="dbg",
                                   name="dbg") if debug else None)

            def dbg(slot, src_fn, rows=128):
                if debug:
                    nc.vector.tensor_copy(
                        dbg_sb[0:rows, slot * 256:(slot + 1) * 256], src_fn())

            # ----- warmup collective: CC rendezvous overlaps LN1 -----
            wu_in = dram.tile([W, 16], f32, name="wu_in")
            wu_out = dram.tile([W, 16], f32, name="wu_out")
            if not mock_cc:
                if sim:
                    nc.sync.dma_start(out=wu_in[:, :], in_=qkb[0:W, 0:16])
                nc.gpsimd.collective_compute(
                    "AllToAll", ALU.bypass, replica_groups=rg,
                    ins=[wu_in.opt()], outs=[wu_out.opt()])

            # ----- constants -----
            ident = const.tile([128, 128], f32, tag="ident", name="ident")
            make_identity(nc, ident[:])
            ident_h = const.tile([128, 128], dt, tag="identh", name="identh")
            nc.vector.tensor_copy(ident_h[:], ident[:])
            eps_sb = const.tile([128, 1], f32, tag="eps", name="eps")
            nc.vector.memset(eps_sb[:], 1e-5)

            cbm_all = const.tile([128, B * CH], dt, tag="cbm", name="cbm")
            dma(cbm_all[:], cbm_in[:, :])

            def load2d(dram_t, shape, name):
                t = const.tile(shape, f32, tag=name, name=name)
                dma(t[:], dram_t[:, :])
                return t

            ln1_w_sb = load2d(ln1_w, [128, 8], "ln1w")
            ln1_b_sb = load2d(ln1_b, [128, 8], "ln1b")
            ln2_w_sb = load2d(ln2_w, [128, 8], "ln2w")
            ln2_b_sb = load2d(ln2_b, [128, 8], "ln2b")
            qkb_sb = load2d(qkb, [128, 24], "qkb")
            proj_b_sb = load2d(proj_b, [128, 8], "projb")
            b1_sb = load2d(b1, [128, 32], "b1")
            b2_sb = load2d(b2, [128, 8], "b2")
            m01_sb = load2d(mask01, [128, NB], "m01")

            # ----- weights: k-half of qkw first (needed first), q-half after
            # (reuses the same slot via tag); wv for the v rows -----
            kw_sb = persist.tile([128, 8 * 8 * 128], dt, tag="qkw", bufs=QKW_BUFS,
                                 name="kw")
            dma(kw_sb[:], qkw[:, 8 * 8 * 128:16 * 8 * 128])  # k blocks

            # ----- LN (row layout) + transpose into [128, 8*CH] bf16 tile -----
            def layer_norm_T(x_tiles, w_sb, b_sb, out_all):
                for t in range(len(x_tiles)):
                    x = x_tiles[t]
                    stat = work.tile([128, 8], f32, tag="lnstat", name="lnstat")
                    scr = work.tile([128, E], dt, tag="lnscr", bufs=LN_BUFS, name="lnscr")
                    nc.vector.reduce_sum(out=stat[:, 0:1], in_=x[:], axis=X)
                    nc.scalar.activation(scr[:], x[:], AF.Square,
                                         accum_out=stat[:, 1:2])
                    nc.vector.tensor_tensor(out=stat[:, 4:5], in0=stat[:, 0:1],
                                            in1=stat[:, 0:1], op=ALU.mult)
                    nc.vector.tensor_scalar(out=stat[:, 4:5], in0=stat[:, 4:5],
                                            scalar1=-1.0 / E,
                                            scalar2=stat[:, 1:2],
                                            op0=ALU.mult, op1=ALU.add)
                    nc.scalar.activation(stat[:, 5:6], stat[:, 4:5], AF.Sqrt,
                                         bias=eps_sb[:], scale=1.0 / E)
                    nc.vector.reciprocal_approx_fast(out=stat[:, 3:4],
                                                     in_=stat[:, 5:6])
                    nc.vector.tensor_scalar(out=stat[:, 6:7], in0=stat[:, 0:1],
                                            scalar1=stat[:, 3:4],
                                            scalar2=-1.0 / E,
                                            op0=ALU.mult, op1=ALU.mult)
                    xn = work.tile([128, E], dt, tag="lnxn", bufs=LN_BUFS, name="lnxn")
                    nc.vector.tensor_scalar(out=xn[:], in0=x[:],
                                            scalar1=stat[:, 3:4],
                                            scalar2=stat[:, 6:7],
                                            op0=ALU.mult, op1=ALU.add)
                    for m in range(8):
                        tp = psum.tile([128, 128], dt, tag="sc", name="tp")
                        nc.tensor.transpose(tp[:], xn[:, m * 128:(m + 1) * 128],
                                            ident_h[:])
                        dst = out_all[:, m * CH + t * 128: m * CH + (t + 1) * 128]
                        if m % 2 == 0:
                            nc.scalar.activation(dst, tp[:], AF.Identity,
                                                 bias=b_sb[:, m:m + 1],
                                                 scale=w_sb[:, m:m + 1])
                        else:
                            nc.vector.tensor_scalar(
                                out=dst, in0=tp[:],
                                scalar1=w_sb[:, m:m + 1],
                                scalar2=b_sb[:, m:m + 1],
                                op0=ALU.mult, op1=ALU.add)

            # ----- stage 1: LN1 on own chunk -----
            x_rows = []
            for t in range(B):
                xt = work.tile([128, E], f32, tag="xrow", bufs=3,
                               name=f"xrow{t}")
                dma(xt[:], hid[t * 128:(t + 1) * 128, :])
                x_rows.append(xt)
            xnT = persist.tile([128, 8 * CH], dt, tag="xnT", name="xnT")
            layer_norm_T(x_rows, ln1_w_sb, ln1_b_sb, xnT)
            dbg(0, lambda: xnT[:, 0:256])

            # ----- stage 2: local qkv (all heads) + A2As -----
            a2a_k_in = dram.tile([W * 128, CH], dt, name="a2a_k_in")
            a2a_k_out = dram.tile([W * 128, CH], dt, name="a2a_k_out")
            a2a_q_in = dram.tile([W * 128, CH], dt, name="a2a_q_in")
            a2a_q_out = dram.tile([W * 128, CH], dt, name="a2a_q_out")
            a2a_v_in = dram.tile([W * 128, CH], dt, name="a2a_v_in")
            a2a_v_out = dram.tile([W * 128, CH], dt, name="a2a_v_out")

            def qk_part(w_sb, boff, a2a_in):
                for m in range(8):
                    ps = psum.tile([128, CH], f32, tag="av", name="qkps")
                    for kb in range(8):
                        nc.tensor.matmul(
                            ps[:],
                            lhsT=w_sb[:, (m * 8 + kb) * 128:(m * 8 + kb + 1) * 128],
                            rhs=xnT[:, kb * CH:(kb + 1) * CH],
                            start=(kb == 0), stop=(kb == 7))
                    sb = work.tile([128, CH], dt, tag="qkloc", bufs=3,
                                   name="qkloc")
                    nc.vector.tensor_scalar(
                        out=sb[:], in0=ps[:],
                        scalar1=qkb_sb[:, boff + m:boff + m + 1],
                        scalar2=None, op0=ALU.add)
                    dma(a2a_in[m * 128:(m + 1) * 128, :], sb[:])

            qk_part(kw_sb, 8, a2a_k_in)
            if mock_cc:
                dma(a2a_k_out[:, :], a2a_k_in[:, :])
            else:
                nc.gpsimd.collective_compute(
                    "AllToAll", ALU.bypass, replica_groups=rg,
                    ins=[a2a_k_in.opt()], outs=[a2a_k_out.opt()])

            qw_sb = persist.tile([128, 8 * 8 * 128], dt, tag="qkw", bufs=QKW_BUFS,
                                 name="qw")
            dma(qw_sb[:], qkw[:, 0:8 * 8 * 128])
            qk_part(qw_sb, 0, a2a_q_in)
            if mock_cc:
                dma(a2a_q_out[:, :], a2a_q_in[:, :])
            else:
                nc.gpsimd.collective_compute(
                    "AllToAll", ALU.bypass, replica_groups=rg,
                    ins=[a2a_q_in.opt()], outs=[a2a_q_out.opt()])

            # v part: same T-layout as k/q (v weight blocks 16..23 reuse slot)
            vw_sb = persist.tile([128, 8 * 8 * 128], dt, tag="qkw",
                                 bufs=QKW_BUFS, name="vw")
            dma(vw_sb[:], qkw[:, 16 * 8 * 128:24 * 8 * 128])
            qk_part(vw_sb, 16, a2a_v_in)
            if mock_cc:
                dma(a2a_v_out[:, :], a2a_v_in[:, :])
            else:
                nc.gpsimd.collective_compute(
                    "AllToAll", ALU.bypass, replica_groups=rg,
                    ins=[a2a_v_in.opt()], outs=[a2a_v_out.opt()])

            # ----- stage 3: assemble kT/qT [128, SS] and v_aug -----
            kT = persist.tile([128, SS], dt, tag="kT", name="kT")
            qT = persist.tile([128, SS], dt, tag="qT", name="qT")
            for s in range(W):
                nc.scalar.dma_start(out=kT[:, s * CH:(s + 1) * CH],
                                    in_=a2a_k_out[s * 128:(s + 1) * 128, :])
                nc.scalar.dma_start(out=qT[:, s * CH:(s + 1) * CH],
                                    in_=a2a_q_out[s * 128:(s + 1) * 128, :])

            # v_aug: [key 128, block*128] rows; cols 0:64 = v dims, 64:128 ones
            # then whole block scaled by external mask (masked keys zero both
            # numerator and denominator => correct softmax masking).
            # Build: assemble vT like kT, PE-transpose each 128-block, fold
            # the mask into the PSUM->SBUF copies.
            vT = work.tile([128, SS], dt, tag="vT", bufs=1, name="vT")
            for s in range(W):
                nc.scalar.dma_start(out=vT[:, s * CH:(s + 1) * CH],
                                    in_=a2a_v_out[s * 128:(s + 1) * 128, :])
            v_aug = [persist.tile([128, NB * 128], dt, tag=f"vaug{h}",
                                  name=f"vaug{h}") for h in range(2)]
            for h in range(2):
                nc.vector.memset(
                    v_aug[h][:].rearrange("p (b c) -> p b c", c=128)[:, :, 64:128],
                    1.0)
            for tb in range(NB):
                tp = psum.tile([128, 128], dt, tag="sc", name="vtp")
                nc.tensor.transpose(tp[:], vT[:, tb * 128:(tb + 1) * 128],
                                    ident_h[:])
                for h in range(2):
                    nc.vector.tensor_scalar(
                        out=v_aug[h][:, tb * 128:tb * 128 + 64],
                        in0=tp[:, 64 * h:64 * h + 64],
                        scalar1=m01_sb[:, tb:tb + 1],
                        scalar2=None, op0=ALU.mult)
                    nc.vector.tensor_scalar(
                        out=v_aug[h][:, tb * 128 + 64:(tb + 1) * 128],
                        in0=v_aug[h][:, tb * 128 + 64:(tb + 1) * 128],
                        scalar1=m01_sb[:, tb:tb + 1],
                        scalar2=None, op0=ALU.mult)

            dbg(1, lambda: kT[:, 0:256])
            dbg(2, lambda: qT[:, 0:256])
            dbg(3, lambda: v_aug[0][:, 0:256])

            # ----- stage 4/5: attention, heads interleaved per group so the
            # PE always has a backlog (other head's scores+AV) while ScalarE
            # runs exp — keeps HAM at full clock.  Single combined out A2A.
            a2a_o_in = dram.tile([W * 128, CH], dt, name="a2a_o_in")
            a2a_o_out = dram.tile([W * 128, CH], dt, name="a2a_o_out")
            for j in range(W):
                n_t = B * (j + 1)
                groups = list(range(0, n_t, 3))
                G = len(groups)
                avs = [psum.tile([128, CH], f32, tag="av", name=f"avacc{h}")
                       for h in range(2)]
                exs = {}
                # scores+exp run one group ahead of AV so the in-order PE
                # queue never waits on ScalarE (software pipeline skew)
                for gi in range(G + 1):
                    if gi < G:
                        g0 = groups[gi]
                        g = min(3, n_t - g0)
                        for h in range(2):
                            e0 = 64 * h
                            sc = psum.tile([128, 3 * 512], f32, tag="sc",
                                           name="sc")
                            for s in range(g):
                                tb = g0 + s
                                if PACK_SCORES:
                                    for hf in range(2):
                                        nc.tensor.matmul(
                                            sc[64 * hf:64 * hf + 64,
                                               s * 512:s * 512 + CH],
                                            lhsT=kT[e0:e0 + 64,
                                                    tb * 128 + 64 * hf:tb * 128 + 64 * hf + 64],
                                            rhs=qT[e0:e0 + 64,
                                                   j * CH:(j + 1) * CH],
                                            start=True, stop=True,
                                            tile_position=(e0, 64 * hf))
                                else:
                                    nc.tensor.matmul(
                                        sc[:, s * 512:s * 512 + CH],
                                        lhsT=kT[e0:e0 + 64,
                                                tb * 128:(tb + 1) * 128],
                                        rhs=qT[e0:e0 + 64,
                                               j * CH:(j + 1) * CH],
                                        start=True, stop=True)
                            ex = exp_pool.tile([128, 3 * CH], dt, tag="ex",
                                               name="ex")
                            if EXP_GROUP:
                                sc3 = sc[:].rearrange("p (s c) -> p s c",
                                                      c=512)
                                ex3 = ex[:].rearrange("p (s c) -> p s c",
                                                      c=CH)
                                nc.scalar.activation(ex3[:, 0:g, 0:CH],
                                                     sc3[:, 0:g, 0:CH],
                                                     AF.Exp)
                            else:
                                for s in range(g):
                                    nc.scalar.activation(
                                        ex[:, s * CH:(s + 1) * CH],
                                        sc[:, s * 512:s * 512 + CH], AF.Exp)
                            if g0 + g == n_t:
                                nc.vector.tensor_tensor(
                                    out=ex[:, (g - B) * CH:g * CH],
                                    in0=ex[:, (g - B) * CH:g * CH],
                                    in1=cbm_all[:], op=ALU.mult)
                            if h == 0 and j == 0 and g0 == 0:
                                dbg(4, lambda ex=ex: ex[:, 0:256])
                            exs[(gi, h)] = ex
                    if gi >= 1:
                        g0p = groups[gi - 1]
                        gp = min(3, n_t - g0p)
                        for h in range(2):
                            ex = exs.pop((gi - 1, h))
                            for s in range(gp):
                                tb = g0p + s
                                nc.tensor.matmul(
                                    avs[h][:],
                                    lhsT=v_aug[h][:, tb * 128:(tb + 1) * 128],
                                    rhs=ex[:, s * CH:(s + 1) * CH],
                                    start=(tb == 0), stop=(tb == n_t - 1))
                for h in range(2):
                    av = avs[h]
                    den = work.tile([64, CH], f32, tag="den", name="den")
                    nc.vector.tensor_copy(den[:], av[64:128, :])
                    rec = work.tile([64, CH], f32, tag="rec", name="rec")
                    nc.vector.reciprocal_approx_fast(out=rec[:], in_=den[:])
                    avn = work.tile([64, CH], dt, tag="avn", name="avn")
                    nc.vector.tensor_tensor(out=avn[:], in0=av[0:64, :],
                                            in1=rec[:], op=ALU.mult)
                    if h == 0 and j == 0:
                        dbg(5, lambda avn=avn: avn[:, 0:256], rows=64)
                    dma(a2a_o_in[j * 128 + 64 * h:j * 128 + 64 * h + 64, :],
                        avn[:])
            if mock_cc:
                dma(a2a_o_out[:, :], a2a_o_in[:, :])
            else:
                nc.gpsimd.collective_compute(
                    "AllToAll", ALU.bypass, replica_groups=rg,
                    ins=[a2a_o_in.opt()], outs=[a2a_o_out.opt()])

            # ----- stage 7: proj + residual -----
            aT = persist.tile([128, SS], dt, tag="aT", name="aT")
            for k in range(W):
                nc.scalar.dma_start(
                    out=aT[:, k * CH:(k + 1) * CH],
                    in_=a2a_o_out[k * 128:(k + 1) * 128, :])
            dbg(6, lambda: aT[:, 0:256])
            res1 = [persist.tile([128, E], f32, tag=f"res1_{t}",
                                 name=f"res1_{t}") for t in range(B)]
            for m in range(8):
                wpm = work.tile([128, 8 * 128], dt, tag="wpm", name="wpm",
                                bufs=WPM_BUFS)
                dma(wpm[:], proj_w[m * 128:(m + 1) * 128, :])
                ps = psum.tile([128, CH], f32, tag="av", name="mmacc")
                for k in range(8):
                    nc.tensor.matmul(
                        ps[:], lhsT=wpm[:, k * 128:(k + 1) * 128],
                        rhs=aT[:, k * CH:(k + 1) * CH],
                        start=(k == 0), stop=(k == 7))
                pTm = work.tile([128, CH], f32, tag="pTm", name="pTm")
                nc.scalar.activation(pTm[:], ps[:], AF.Identity,
                                     bias=proj_b_sb[:, m:m + 1], scale=1.0)
                for t in range(B):
                    xr = work.tile([128, 128], f32, tag="xres", bufs=4,
                                   name="xres")
                    dma(xr[:], hid[t * 128:(t + 1) * 128,
                                   m * 128:(m + 1) * 128])
                    tp = psum.tile([128, 128], f32, tag="sc", name="tp")
                    nc.tensor.transpose(tp[:], pTm[:, t * 128:(t + 1) * 128],
                                        ident[:])
                    nc.vector.tensor_tensor(
                        out=res1[t][:, m * 128:(m + 1) * 128],
                        in0=tp[:], in1=xr[:], op=ALU.add)

            # ----- stage 8: LN2 -----
            dbg(7, lambda: res1[0][:, 0:256])
            l2T = persist.tile([128, 8 * CH], dt, tag="l2T", name="l2T")
            layer_norm_T(res1, ln2_w_sb, ln2_b_sb, l2T)
            dbg(8, lambda: l2T[:, 0:256])

            # ----- stage 9: MLP -----
            scratch = persist.tile([128, 40 * CH], dt, tag="scratch",
                                   name="scratch")
            h1T = [scratch[:, m * CH:(m + 1) * CH] for m in range(32)]
            for m in range(32):
                w1m = work.tile([128, E], dt, tag="w1m", name="w1m", bufs=W1M_BUFS)
                dma(w1m[:], w1[m * 128:(m + 1) * 128, :])
                ps = psum.tile([128, CH], f32, tag="av", name="mmacc")
                for k in range(8):
                    nc.tensor.matmul(
                        ps[:], lhsT=w1m[:, k * 128:(k + 1) * 128],
                        rhs=l2T[:, k * CH:(k + 1) * CH],
                        start=(k == 0), stop=(k == 7))
                nc.scalar.activation(h1T[m], ps[:], AF.Relu,
                                     bias=b1_sb[:, m:m + 1], scale=1.0)

            oT = [scratch[:, (32 + m) * CH:(33 + m) * CH] for m in range(8)]
            for m in range(8):
                ps = psum.tile([128, CH], f32, tag="av", name="mmacc")
                for half in range(2):
                    w2m = work.tile([128, 16 * 128], dt, tag="w2m", name="w2m",
                                    bufs=W2M_BUFS)
                    dma(w2m[:], w2[m * 128:(m + 1) * 128,
                                   half * 16 * 128:(half + 1) * 16 * 128])
                    for k in range(16):
                        nc.tensor.matmul(
                            ps[:], lhsT=w2m[:, k * 128:(k + 1) * 128],
                            rhs=h1T[half * 16 + k],
                            start=(half == 0 and k == 0),
                            stop=(half == 1 and k == 15))
                nc.scalar.activation(oT[m], ps[:], AF.Identity,
                                     bias=b2_sb[:, m:m + 1], scale=1.0)

            # ----- stage 10: transpose back + final residual + out -----
            for t in range(B):
                orow = work.tile([128, E], f32, tag="orow", bufs=OROW_BUFS, name="orow")
                for m in range(8):
                    tp = psum.tile([128, 128], dt, tag="sc", name="tpo")
                    nc.tensor.transpose(tp[:], oT[m][:, t * 128:(t + 1) * 128],
                                        ident_h[:])
                    nc.vector.tensor_tensor(
                        out=orow[:, m * 128:(m + 1) * 128],
                        in0=tp[:], in1=res1[t][:, m * 128:(m + 1) * 128],
                        op=ALU.add)
                dma(out[t * 128:(t + 1) * 128, :], orow[:])
            if debug:
                dbg(9, lambda: oT[0][:, 0:256])
                dma(dbg_t[:, :], dbg_sb[:])

    return nc


def _cbm(CH, wdt):
    Bv = CH // 128
    t = np.arange(128)[:, None]
    s = np.arange(CH)[None, :]
    cb = np.zeros((128, Bv * CH), np.float32)
    for p in range(Bv):
        cb[:, p * CH:(p + 1) * CH] = (s - t - 128 * p >= 0)
    return np.ascontiguousarray(cb.astype(wdt))


def _prepare_in_maps(inputs, SS: int, dt_name: str = "bfloat16"):
    """Host-side prep: slice hidden per core; pre-tile weight matrices so
    every device DMA is contiguous; cast mm weights to dt; prescale q 1/8."""
    import ml_dtypes

    wdt = ml_dtypes.bfloat16 if dt_name == "bfloat16" else np.float32
    CH = SS // W
    NB = SS // 128
    hid = np.ascontiguousarray(
        np.asarray(inputs["hidden_states"], np.float32)[0, :SS])
    attn_w = np.asarray(inputs["attn_w"], np.float32).copy()
    attn_b = np.asarray(inputs["attn_b"], np.float32).copy()
    attn_w[:, :E] *= 0.125
    attn_b[:E] *= 0.125
    mask = np.asarray(inputs["mask"])[0, 0, 0, :SS]
    mask01 = mask.astype(np.float32)

    def vec2d(v, n):
        return np.ascontiguousarray(
            np.asarray(v, np.float32)[:n].reshape(n // 128, 128).T)

    proj_w = np.asarray(inputs["proj_w"], np.float32)
    w1 = np.asarray(inputs["mlp_w1"], np.float32)
    w2 = np.asarray(inputs["mlp_w2"], np.float32)

    # X[k*128+p, m*128+f] -> [(m p), (k f)]
    def tile_mk(x, km, mm_):
        return np.ascontiguousarray(
            x.reshape(km, 128, mm_, 128).transpose(2, 1, 0, 3)
            .reshape(mm_ * 128, km * 128))

    # q,k,v stationary blocks: qkw[:, (m*8+kb)*128 : +128] =
    #   attn_w[kb*128:(kb+1)*128, c0 + m'*128 : +128]
    # m 0..7 = q (prescaled 1/8), 8..15 = k, 16..23 = v
    qkw = np.empty((128, 24, 8, 128), np.float32)
    qkb = np.empty((128, 24), np.float32)
    for m in range(24):
        c0 = (m // 8) * E
        mm = m % 8
        for kb in range(8):
            qkw[:, m, kb, :] = attn_w[kb * 128:(kb + 1) * 128,
                                      c0 + mm * 128:c0 + (mm + 1) * 128]
        qkb[:, m] = attn_b[c0 + mm * 128:c0 + (mm + 1) * 128]

    common = {
        "qkw": np.ascontiguousarray(qkw.reshape(128, -1)).astype(wdt),
        "qkb": np.ascontiguousarray(qkb),
        "mask01": np.ascontiguousarray(mask01.reshape(NB, 128).T),
        "proj_w": tile_mk(proj_w, 8, 8).astype(wdt),
        "proj_b": vec2d(inputs["proj_b"], E),
        "ln1_w": vec2d(inputs["ln1_w"], E),
        "ln1_b": vec2d(inputs["ln1_b"], E),
        "ln2_w": vec2d(inputs["ln2_w"], E),
        "ln2_b": vec2d(inputs["ln2_b"], E),
        "w1": tile_mk(w1, 8, 32).astype(wdt),
        "b1": vec2d(inputs["mlp_b1"], I),
        "w2": tile_mk(w2, 32, 8).astype(wdt),
        "b2": vec2d(inputs["mlp_b2"], E),
        "cbm": _cbm(CH, wdt),
    }
    in_maps = []
    for i in range(W):
        in_maps.append({
            "hidden": np.ascontiguousarray(hid[i * CH:(i + 1) * CH]),
            **common,
        })
    return in_maps


def _run(inputs, SS, dt_name="bfloat16", **kw):
    from concourse.bass_utils import run_bass_kernel_spmd

    key = (SS, dt_name)
    if key not in _CACHE:
        nc = _build(SS, dt_name)
        nc.finalize()
        _CACHE[key] = nc
    nc = _CACHE[key]
    in_maps = _prepare_in_maps(inputs, SS, dt_name)
    res = run_bass_kernel_spmd(nc, in_maps, core_ids=list(range(W)), **kw)
    full = np.concatenate([r["out"] for r in res.results], axis=0)
    return full[None].astype(np.float32), res


def kernel(**inputs) -> np.ndarray:
    out, _ = _run(inputs, 3072, "bfloat16")
    return out
